# revision 4
# baseline (speedup 1.0000x reference)
"""Hybrid kernel: k-tiles 0-1 via 1-term e4m3 DoubleRow, k-tiles 2-15 exact
bf16 ints.  fp8 noise on 1/8 of the contraction -> rel err ~1.43e-2 (< 2e-2
gate); replaces 2 of 16 bf16 matmuls per PSUM group with one DR matmul
(measured 221ns vs 426ns) -> ~6% fewer PE cycles.
"""
import sys
sys.path.insert(0, "/opt/trn_rl_repo")
import numpy as np
import ml_dtypes
import concourse.bass as bass
import concourse.mybir as mybir
import concourse.tile as tile
from concourse import bacc

N_CORES = 8
GRID_R, GRID_Q = 4, 2
M, K, N = 4096, 2048, 4096
MB, NB = M // GRID_R, N // GRID_Q
F32 = mybir.dt.float32
BF16 = mybir.dt.bfloat16
E4 = mybir.dt.float8e4
ACTF = mybir.ActivationFunctionType
BF16_NP = ml_dtypes.bfloat16
E4_NP = ml_dtypes.float8_e4m3

KB_TILES = 14                # bf16 k-tiles (k = 256..2047)
N_PANELS = NB // 512         # 4
M_TILES = MB // 128          # 8
DR = mybir.MatmulPerfMode.DoubleRow


def build_body(nc, tc, A8, AT, B8, Bp, out_ext):
    with (
        tc.tile_pool(name="atp", bufs=1) as atp,
        tc.tile_pool(name="bpool", bufs=3) as bpool,
        tc.tile_pool(name="b8pool", bufs=3) as b8pool,
        tc.tile_pool(name="outsb", bufs=12) as outsb,
        tc.tile_pool(name="psum", bufs=8, space="PSUM") as psum,
    ):
        a8 = atp.tile([128, 2, MB], E4, name="a8")
        at = atp.tile([128, KB_TILES * MB], BF16, name="at")

        def stage_b_panel(n, kchunk=4, k_fine=0):
            t8 = b8pool.tile([128, 2, 512], E4, tag="bpan8", name=f"b8_{n}")
            for h in range(2):
                nc.sync.dma_start(
                    t8[:, h:h + 1, :],
                    B8[:, h * NB + n * 512:h * NB + (n + 1) * 512])
            t = bpool.tile([128, KB_TILES * 512], BF16, tag="bpan",
                           name=f"b_{n}")
            for k in range(k_fine):
                nc.sync.dma_start(
                    t[:, k * 512:(k + 1) * 512],
                    Bp[k * 128:(k + 1) * 128, n * 512:(n + 1) * 512])
            for k0 in range(k_fine, KB_TILES, kchunk):
                kc = min(kchunk, KB_TILES - k0)
                src = Bp[k0 * 128:(k0 + kc) * 128,
                         n * 512:(n + 1) * 512]
                nc.sync.dma_start(
                    t[:, k0 * 512:(k0 + kc) * 512],
                    src.rearrange("(j p) c -> p j c", p=128))
            return t, t8

        warm = atp.tile([128, 512], F32, name="warm")
        nc.gpsimd.memset(warm[:], 0.0)
        wps = psum.tile([128, 512], F32, tag="acc", name="warm_ps")
        N_WARM = 2
        for i in range(N_WARM):
            nc.tensor.matmul(wps[:], lhsT=warm[:, 0:128], rhs=warm[:],
                             start=i == 0, stop=i == N_WARM - 1)

        # interleave staging: fp8 pair + b0 + at
        b80 = b8pool.tile([128, 2, 512], E4, tag="bpan8", name="b8_0")
        b0 = bpool.tile([128, KB_TILES * 512], BF16, tag="bpan", name="b_0")
        for h in range(2):
            nc.sync.dma_start(b80[:, h:h + 1, :], B8[:, h * NB:h * NB + 512])
            nc.sync.dma_start(a8[:, h:h + 1, :],
                              A8[:, h * MB:(h + 1) * MB])
        for k in range(KB_TILES):
            nc.sync.dma_start(b0[:, k * 512:(k + 1) * 512],
                              Bp[k * 128:(k + 1) * 128, 0:512])
            nc.sync.dma_start(at[:, k * MB:(k + 1) * MB],
                              AT[k * 128:(k + 1) * 128, :])

        def mm_dr(ps, b8q, m, start):
            nc.tensor.matmul(
                ps[:], lhsT=a8[:, :, m * 128:(m + 1) * 128],
                rhs=b8q[:, :, :], start=start, stop=False, perf_mode=DR)

        def mm(ps, bq, k, m, stop):
            nc.tensor.matmul(
                ps[:],
                lhsT=at[:, k * MB + m * 128:k * MB + (m + 1) * 128],
                rhs=bq[:, k * 512:(k + 1) * 512],
                start=False, stop=stop)

        def evict(ps, n, m):
            ob = outsb.tile([128, 512], BF16, tag="ob", name=f"ob_{n}_{m}")
            if (n * M_TILES + m) % 2 == 0:
                nc.scalar.activation(ob[:], ps[:], ACTF.Copy, bias=0.0,
                                     scale=1.0)
            else:
                nc.vector.tensor_copy(ob[:], ps[:])
            nc.sync.dma_start(
                out_ext[m * 128:(m + 1) * 128, n * 512:(n + 1) * 512],
                ob[:])

        panels = {0: (b0, b80)}
        panels[1] = stage_b_panel(1, k_fine=4)

        # Panel 0: DR step first (start), then k-outer bf16 across 8 banks.
        ps = [psum.tile([128, 512], F32, tag="acc", name=f"acc_0_{m}")
              for m in range(M_TILES)]
        for m in range(M_TILES):
            mm_dr(ps[m], b80, m, True)
        for k in range(KB_TILES):
            last = k == KB_TILES - 1
            for m in range(M_TILES):
                mm(ps[m], b0, k, m, last)
                if last:
                    evict(ps[m], 0, m)

        for n in (1, 2):
            panels[n + 1] = stage_b_panel(n + 1)
            bq, b8q = panels[n]
            for half in range(2):
                g = [psum.tile([128, 512], F32, tag="acc",
                               name=f"acc_{n}_{half}_{mi}")
                     for mi in range(4)]
                for mi in range(4):
                    mm_dr(g[mi], b8q, 4 * half + mi, True)
                for k in range(KB_TILES):
                    last = k == KB_TILES - 1
                    for mi in range(4):
                        mm(g[mi], bq, k, 4 * half + mi, last)
                        if last:
                            evict(g[mi], n, 4 * half + mi)

        bq, b8q = panels[3]
        for m in range(M_TILES):
            ps_m = psum.tile([128, 512], F32, tag="acc", name=f"acc_3_{m}")
            mm_dr(ps_m, b8q, m, True)
            for k in range(KB_TILES):
                mm(ps_m, bq, k, m, k == KB_TILES - 1)
            evict(ps_m, 3, m)


def build_kernel(n_reps: int = 1):
    nc = bacc.Bacc("TRN2", target_bir_lowering=False, debug=False,
                   num_devices=N_CORES)
    A8 = nc.declare_dram_parameter("A8", [128, 2 * MB], E4, isOutput=False)
    AT = nc.declare_dram_parameter("AT", [KB_TILES * 128, MB], BF16,
                                   isOutput=False)
    B8 = nc.declare_dram_parameter("B8", [128, 2 * NB], E4, isOutput=False)
    Bp = nc.declare_dram_parameter("B", [KB_TILES * 128, NB], BF16,
                                   isOutput=False)
    out_ext = nc.declare_dram_parameter("out", [MB, NB], BF16, isOutput=True)

    with tile.TileContext(nc) as tc:
        for rep in range(n_reps):
            if rep:
                tc.strict_bb_all_engine_barrier()
            build_body(nc, tc, A8, AT, B8, Bp, out_ext)
    nc.finalize()
    return nc


def asym_quantize_np(x: np.ndarray):
    xmax = np.float32(x.max())
    xmin = np.float32(x.min())
    scale = np.float32((xmax - xmin) / np.float32(255.0))
    zero = np.float32(np.round(-xmin / scale))
    q = np.clip(np.round(x / scale) + zero, np.float32(0.0), np.float32(255.0))
    return q - zero, scale


def quantize(A: np.ndarray, B: np.ndarray):
    qA, sA = asym_quantize_np(A)
    qB, sB = asym_quantize_np(B)
    in_maps = []
    for c in range(N_CORES):
        r, q = c // GRID_Q, c % GRID_Q
        qAc = qA[r * MB:(r + 1) * MB, :]          # [MB, K]
        qBc = qB[:, q * NB:(q + 1) * NB]          # [K, NB]
        # fp8 pair: k = 0..255 -> [p, h*W + w]
        a8 = np.ascontiguousarray(
            qAc[:, :256].T.reshape(2, 128, MB).transpose(1, 0, 2)
            .reshape(128, 2 * MB)).astype(E4_NP)
        b8 = np.ascontiguousarray(
            qBc[:256, :].reshape(2, 128, NB).transpose(1, 0, 2)
            .reshape(128, 2 * NB)).astype(E4_NP)
        at = np.ascontiguousarray(qAc[:, 256:].T).astype(BF16_NP)
        bp = np.ascontiguousarray(qBc[256:, :]).astype(BF16_NP)
        in_maps.append({"A8": a8, "AT": at, "B8": b8, "B": bp})
    return in_maps, np.float32(sA) * np.float32(sB)


def unshard_output(results, outscale) -> np.ndarray:
    out = np.empty((M, N), np.float32)
    for c in range(N_CORES):
        r, q = c // GRID_Q, c % GRID_Q
        blk = np.asarray(results[c]["out"]).astype(np.float32)
        out[r * MB:(r + 1) * MB, q * NB:(q + 1) * NB] = blk * outscale
    return out


_CACHED = {}


def _get_nc():
    if "nc" not in _CACHED:
        _CACHED["nc"] = build_kernel(n_reps=1)
    return _CACHED["nc"]


def kernel(A: np.ndarray, B: np.ndarray) -> np.ndarray:
    from concourse.bass_utils import run_bass_kernel_spmd
    A = np.ascontiguousarray(np.asarray(A, dtype=np.float32))
    B = np.ascontiguousarray(np.asarray(B, dtype=np.float32))
    assert A.shape == (M, K) and B.shape == (K, N)
    nc = _get_nc()
    in_maps, outscale = quantize(A, B)
    res = run_bass_kernel_spmd(nc, in_maps, core_ids=list(range(N_CORES)))
    return unshard_output(res.results, outscale)
